# revision 3
# baseline (speedup 1.0000x reference)
"""DiagonalwiseSeparableLayer on 8 Trainium2 cores — final.

Strategy vs baseline:
  - Host pre-transposes x to channel-major padded layout, bf16:
    [img, half(128ch), 114*114 grid] with zero borders (stride 114).
    Device input phase = pure DMA (no PE transposes, no DVE casts).
  - Output written channel-major bf16 [img, co, 12544 px]; host
    transposes back to NHWC fp32. Halves HBM write traffic.
  - DW 3x3 depthwise: 64x64 PE tile mode, 4 tiles = 2 chunks (456 px
    = 4 rows) concurrently; tap-outer loop over 2 chunk-pairs reuses
    each diag stationary for 4 matmuls. Odd chunks land partition-
    swapped in psum; consumed as-is by a row-swapped PW stationary.
  - PW 1x1 grouped conv: full-array matmul, stationary = block-diag
    pw weights [128 ci, 128 co] (2 per half x 2 parity), moving = y.
    Output [co, px] direct -> compacting psum->sbuf bf16 copy -> DMA.
"""
import numpy as np
import ml_dtypes
from contextlib import ExitStack

import concourse.bacc as bacc
import concourse.tile as tile
from concourse import mybir
from concourse.bass_utils import run_bass_kernel_spmd

N_CORES = 8
B, H, W, CIN, COUT = 16, 112, 112, 256, 512
G, CPG = 8, 32
BPC = B // N_CORES            # images per core
RS = 114                      # padded row stride
NPR = 114                     # padded rows
XLEN = RS * NPR               # 12996 cols per channel
GUARD = 64
CHUNK = 4 * RS                # 456 px (4 padded rows) per matmul chunk
NCH = H // 4                  # 28 chunks per half-image
SCN = NCH // 4                # 7 super-chunks (4 chunks each)
VPX = H * W                   # 12544 valid px per image
BF16 = mybir.dt.bfloat16
F32 = mybir.dt.float32

_CACHE = {}


def _build(num_devices=N_CORES):
    nc = bacc.Bacc("TRN2", target_bir_lowering=False, debug=False,
                   num_devices=num_devices)
    x_d = nc.dram_tensor("x", [BPC, 2, 128, XLEN], BF16,
                         kind="ExternalInput").ap()
    wdw_d = nc.dram_tensor("w", [128, 2 * 9 * 64], BF16,
                           kind="ExternalInput").ap()
    wpw_d = nc.dram_tensor("wp", [128, 2 * 2 * 2 * 128], BF16,
                           kind="ExternalInput").ap()
    out_d = nc.dram_tensor("out", [BPC, COUT, VPX], BF16,
                           kind="ExternalOutput").ap()

    xg = GUARD + XLEN + GUARD

    with tile.TileContext(nc) as tc, ExitStack() as ctx:
        const = ctx.enter_context(tc.tile_pool(name="const", bufs=1))
        xch_pool = ctx.enter_context(tc.tile_pool(name="xch", bufs=4))
        y_pool = ctx.enter_context(tc.tile_pool(name="ysb", bufs=2))
        stg_pool = ctx.enter_context(tc.tile_pool(name="stg", bufs=6))
        ps_dw = ctx.enter_context(tc.tile_pool(name="psdw", bufs=4,
                                               space="PSUM"))
        ps_pw = ctx.enter_context(tc.tile_pool(name="pspw", bufs=4,
                                               space="PSUM"))

        wdw_sb = const.tile([128, 2 * 9 * 64], BF16)
        nc.scalar.dma_start(out=wdw_sb[:], in_=wdw_d[:])
        wpw_sb = const.tile([128, 2 * 2 * 2 * 128], BF16)
        nc.scalar.dma_start(out=wpw_sb[:], in_=wpw_d[:])

        # ---- PE warmup: keep HAM busy while input DMA streams ----
        warm_w = const.tile([128, 512], BF16)
        nc.gpsimd.memset(warm_w[:], 0.0)
        warm_ps = ps_pw.tile([128, 512], F32, tag="pspw", name="warm")
        for i in range(24):
            nc.tensor.matmul(warm_ps[:], warm_w[:, 0:128], warm_w[:],
                             start=True, stop=True, skip_group_check=True,
                             tile_position=(0, 0))

        # ---- input DMA: first 2 half-images upfront on sync queue,
        # the rest staggered on the ACT queue mid-loop ----
        NPAIR = NCH // 2          # 14 pairs per half-image
        HIMGS = [(b, hh) for b in range(BPC) for hh in range(2)]
        xh = {}

        def load_x(b, hh, eng, eng2=None):
            t = xch_pool.tile([128, xg], BF16, tag="xch",
                              name=f"xch_{b}_{hh}")
            nc.gpsimd.memset(t[:, 0:GUARD], 0.0)
            nc.gpsimd.memset(t[:, GUARD + XLEN:xg], 0.0)
            qc = XLEN // 4
            for q in range(4):
                e = eng if (eng2 is None or q % 2 == 0) else eng2
                e.dma_start(
                    out=t[:, GUARD + q * qc:GUARD + (q + 1) * qc],
                    in_=x_d[b, hh, :, q * qc:(q + 1) * qc])
            xh[(b, hh)] = t

        load_x(*HIMGS[0], nc.sync, nc.scalar)
        load_x(*HIMGS[1], nc.sync, nc.scalar)

        # ---- compute: pipeline unit = chunk pair (2 chunks, 912 px) ----
        # PE stream software-pipelined: PW of pair k is emitted after DW
        # of pair k+1, so the PE never waits on the y psum->sbuf copies.
        def dw_pair(b, hh, pr):
            xt = xh[(b, hh)]
            pdw = [ps_dw.tile([128, CHUNK], F32, tag="psdw",
                              name=f"dw_{b}_{hh}_{pr}_{c}")
                   for c in range(2)]
            ce = 2 * pr
            for t in range(9):
                ty, tx = t // 3 - 1, t % 3 - 1
                dt = ty * RS + tx
                for ch2 in range(2):             # channel 64-half
                    wsl = wdw_sb[64 * ch2:64 * ch2 + 64,
                                 (hh * 9 + t) * 64:
                                 (hh * 9 + t) * 64 + 64]
                    for par in range(2):         # chunk parity
                        c = ce + par
                        w0 = GUARD + (4 * c + 1) * RS + dt
                        jj = ch2 if par == 0 else 1 - ch2
                        nc.tensor.matmul(
                            pdw[par][64 * jj:64 * jj + 64, :],
                            wsl,
                            xt[64 * ch2:64 * ch2 + 64, w0:w0 + CHUNK],
                            start=(t == 0), stop=(t == 8),
                            skip_group_check=True,
                            tile_position=(64 * ch2, 64 * jj),
                        )
            # y copies (DVE/ACT queues; PE does not wait here)
            ysb = y_pool.tile([128, 2 * CHUNK], BF16, tag="ysb",
                              name=f"ysb_{b}_{hh}_{pr}")
            nc.vector.tensor_copy(ysb[:, 0:CHUNK], pdw[0][:])
            nc.scalar.copy(ysb[:, CHUNK:2 * CHUNK], pdw[1][:])
            return ysb

        # staging covers 2 pairs (one super-chunk) for 3584B DMA runs
        stg_cur = {}

        def pw_pair(b, hh, pr, ysb):
            if pr % 2 == 0:
                stg_cur[0] = [stg_pool.tile([128, 4 * 448], BF16,
                                            tag="stg",
                                            name=f"stg_{b}_{hh}_{pr}_{c}")
                              for c in range(2)]
            stg = stg_cur[0]
            po = (pr % 2) * 2          # pair offset in staging (chunks)
            for par in range(2):
                for co2 in range(2):
                    ppw = ps_pw.tile([128, CHUNK], F32, tag="pspw",
                                     name=f"pw_{b}_{hh}_{pr}_{par}_{co2}")
                    nc.tensor.matmul(
                        ppw[:],
                        wpw_sb[:, ((par * 2 + hh) * 2 + co2) * 128:
                               ((par * 2 + hh) * 2 + co2) * 128 + 128],
                        ysb[:, par * CHUNK:(par + 1) * CHUNK],
                        start=True, stop=True,
                        skip_group_check=True,
                        tile_position=(0, 0),
                    )
                    psrc = ppw[:].rearrange("p (r w) -> p r w", w=RS)
                    pdst = stg[co2][:, (po + par) * 448:
                                    (po + par + 1) * 448
                                    ].rearrange("p (r w) -> p r w", w=112)
                    if (par + co2) % 2 == 0:
                        nc.vector.tensor_copy(pdst, psrc[:, :, 1:113])
                    else:
                        nc.scalar.copy(pdst, psrc[:, :, 1:113])
            last_sc = (b, hh) == HIMGS[-1] and pr >= NPAIR - 2
            if last_sc:
                # tail trim: flush per pair so the final DMAs start early
                sc = pr // 2
                for co2 in range(2):
                    nc.sync.dma_start(
                        out=out_d[b, hh * 256 + co2 * 128:
                                  hh * 256 + co2 * 128 + 128,
                                  sc * 1792 + po * 448:
                                  sc * 1792 + (po + 2) * 448],
                        in_=stg[co2][:, po * 448:(po + 2) * 448])
            elif pr % 2 == 1:
                sc = pr // 2
                for co2 in range(2):
                    nc.sync.dma_start(
                        out=out_d[b, hh * 256 + co2 * 128:
                                  hh * 256 + co2 * 128 + 128,
                                  sc * 1792:(sc + 1) * 1792],
                        in_=stg[co2][:])

        work = [(b, hh, pr) for (b, hh) in HIMGS for pr in range(NPAIR)]
        prev = None
        for wi, (b, hh, pr) in enumerate(work):
            if pr == 5 and (b, hh) != HIMGS[-1]:
                nxt = HIMGS[HIMGS.index((b, hh)) + 1]
                if nxt not in xh:
                    load_x(*nxt, nc.scalar)
            ysb = dw_pair(b, hh, pr)
            if prev is not None:
                pw_pair(*prev)
            prev = (b, hh, pr, ysb)
        pw_pair(*prev)

    nc.compile()
    return nc


def _prep_x(x):
    """(B,H,W,256) fp32 -> (B, 2, 128, 114*114) bf16 padded ch-major."""
    xb = np.asarray(x, dtype=np.float32).astype(ml_dtypes.bfloat16)
    xp = np.zeros((B, 2, 128, NPR, RS), dtype=ml_dtypes.bfloat16)
    xp[:, :, :, 1:113, 1:113] = xb.transpose(0, 3, 1, 2).reshape(
        B, 2, 128, H, W)
    return np.ascontiguousarray(xp.reshape(B, 2, 128, XLEN))


def _prep_weights(splitw, pw):
    sw = np.asarray(splitw, dtype=np.float64)
    pwf = np.asarray(pw, dtype=np.float64).reshape(CPG, COUT)
    diag = sw[:, :, :, np.arange(CPG), np.arange(CPG)]   # (G,3,3,ci)
    # DW: [128 rows, (hh, tap) x 64] diag within each 64-block
    wdw = np.zeros((128, 2 * 9 * 64), dtype=np.float64)
    for p in range(128):
        for hh in range(2):
            ch = hh * 128 + p
            g, ci = ch // 32, ch % 32
            for t in range(9):
                wdw[p, (hh * 9 + t) * 64 + (p % 64)] = \
                    diag[g, t // 3, t % 3, ci]
    # PW: [128 rows, (parity, hh, co2) x 128]
    wpw = np.zeros((128, 2 * 2 * 2 * 128), dtype=np.float64)
    for par in range(2):
        for hh in range(2):
            for co2 in range(2):
                blk = ((par * 2 + hh) * 2 + co2) * 128
                for p in range(128):
                    prow = p if par == 0 else (p + 64) % 128
                    ci_g = hh * 128 + prow
                    for m in range(128):
                        co_g = hh * 256 + co2 * 128 + m
                        if ci_g // 32 == co_g // 64:
                            wpw[p, blk + m] = pwf[ci_g % 32, co_g]
    return (wdw.astype(ml_dtypes.bfloat16), wpw.astype(ml_dtypes.bfloat16))


def _post_out(res_list):
    """8 x (BPC, 512, 12544) bf16 -> (B,H,W,512) fp32."""
    o = np.concatenate(res_list, axis=0)           # (B, 512, 12544)
    return np.ascontiguousarray(
        o.astype(np.float32).transpose(0, 2, 1)).reshape(B, H, W, COUT)


def _in_maps(inputs):
    xt = _prep_x(inputs["x"])
    wdw_arr, wpw_arr = _prep_weights(inputs["splitw"], inputs["pw"])
    return [{"x": xt[i * BPC:(i + 1) * BPC], "w": wdw_arr,
             "wp": wpw_arr}
            for i in range(N_CORES)]


def kernel(x, splitw, pw):
    in_maps = _in_maps({"x": x, "splitw": splitw, "pw": pw})
    if "nc" not in _CACHE:
        _CACHE["nc"] = _build()
    nc = _CACHE["nc"]
    res = run_bass_kernel_spmd(nc, in_maps, list(range(N_CORES)))
    return _post_out([res.results[i]["out"] for i in range(N_CORES)])


# revision 4
# speedup vs baseline: 1.1485x; 1.1485x over previous
"""DiagonalwiseSeparableLayer on 8 Trainium2 cores — final.

Strategy vs baseline:
  - Host pre-transposes x to channel-major padded layout, bf16:
    [img, half(128ch), 114*114 grid] with zero borders (stride 114).
    Device input phase = pure DMA (no PE transposes, no DVE casts).
  - Output written channel-major bf16 [img, co, 12544 px]; host
    transposes back to NHWC fp32. Halves HBM write traffic.
  - DW 3x3 depthwise: 64x64 PE tile mode, 4 tiles = 2 chunks (456 px
    = 4 rows) concurrently; tap-outer loop over a chunk pair. Odd
    chunks land partition-swapped in psum; consumed as-is by a
    row-swapped PW stationary (no extra copies).
  - PW 1x1 grouped conv: full-array matmul, stationary = block-diag
    pw weights [128 ci, 128 co] (2 per half x 2 parity), moving = y.
    Output [co, px] direct -> compacting psum->sbuf bf16 copy -> DMA.
  - Pipeline: unit = chunk pair; the PE stream is software-pipelined
    (PW of pair k emitted after DW of pair k+1) so PE never waits on
    psum evacuations. ~25 warmup matmuls pre-warm the HAM clock gate
    during the input DMA window.
  - DMA: input on both hardware DGE rings (sync + scalar queues,
    interleaved quarters; later half-images prefetched mid-loop) so
    output DMAs on the sync ring never queue behind input; staging
    flushed per super-chunk (3584B runs), per-pair for the final one.
"""
import numpy as np
import ml_dtypes
from contextlib import ExitStack

import concourse.bacc as bacc
import concourse.tile as tile
from concourse import mybir
from concourse.bass_utils import run_bass_kernel_spmd

N_CORES = 8
B, H, W, CIN, COUT = 16, 112, 112, 256, 512
G, CPG = 8, 32
BPC = B // N_CORES            # images per core
RS = 114                      # padded row stride
NPR = 114                     # padded rows
XLEN = RS * NPR               # 12996 cols per channel
GUARD = 64
CHUNK = 4 * RS                # 456 px (4 padded rows) per matmul chunk
NCH = H // 4                  # 28 chunks per half-image
SCN = NCH // 4                # 7 super-chunks (4 chunks each)
VPX = H * W                   # 12544 valid px per image
BF16 = mybir.dt.bfloat16
F32 = mybir.dt.float32

_CACHE = {}


def _build(num_devices=N_CORES):
    nc = bacc.Bacc("TRN2", target_bir_lowering=False, debug=False,
                   num_devices=num_devices)
    x_d = nc.dram_tensor("x", [BPC, 2, 128, XLEN], BF16,
                         kind="ExternalInput").ap()
    wdw_d = nc.dram_tensor("w", [128, 2 * 9 * 64], BF16,
                           kind="ExternalInput").ap()
    wpw_d = nc.dram_tensor("wp", [128, 2 * 2 * 2 * 128], BF16,
                           kind="ExternalInput").ap()
    out_d = nc.dram_tensor("out", [BPC, COUT, VPX], BF16,
                           kind="ExternalOutput").ap()

    xg = GUARD + XLEN + GUARD

    with tile.TileContext(nc) as tc, ExitStack() as ctx:
        const = ctx.enter_context(tc.tile_pool(name="const", bufs=1))
        xch_pool = ctx.enter_context(tc.tile_pool(name="xch", bufs=4))
        y_pool = ctx.enter_context(tc.tile_pool(name="ysb", bufs=2))
        stg_pool = ctx.enter_context(tc.tile_pool(name="stg", bufs=6))
        ps_dw = ctx.enter_context(tc.tile_pool(name="psdw", bufs=4,
                                               space="PSUM"))
        ps_pw = ctx.enter_context(tc.tile_pool(name="pspw", bufs=4,
                                               space="PSUM"))

        wdw_sb = const.tile([128, 2 * 9 * 64], BF16)
        nc.scalar.dma_start(out=wdw_sb[:], in_=wdw_d[:])
        wpw_sb = const.tile([128, 2 * 2 * 2 * 128], BF16)
        nc.scalar.dma_start(out=wpw_sb[:], in_=wpw_d[:])

        # ---- PE warmup: keep HAM busy while input DMA streams ----
        warm_w = const.tile([128, 512], BF16)
        nc.gpsimd.memset(warm_w[:], 0.0)
        warm_ps = ps_pw.tile([128, 512], F32, tag="pspw", name="warm")
        for i in range(24):
            nc.tensor.matmul(warm_ps[:], warm_w[:, 0:128], warm_w[:],
                             start=True, stop=True, skip_group_check=True,
                             tile_position=(0, 0))

        # ---- input DMA: first 2 half-images upfront on sync queue,
        # the rest staggered on the ACT queue mid-loop ----
        NPAIR = NCH // 2          # 14 pairs per half-image
        HIMGS = [(b, hh) for b in range(BPC) for hh in range(2)]
        xh = {}

        def load_x(b, hh, eng, eng2=None):
            t = xch_pool.tile([128, xg], BF16, tag="xch",
                              name=f"xch_{b}_{hh}")
            nc.gpsimd.memset(t[:, 0:GUARD], 0.0)
            nc.gpsimd.memset(t[:, GUARD + XLEN:xg], 0.0)
            qc = XLEN // 4
            for q in range(4):
                e = eng if (eng2 is None or q % 2 == 0) else eng2
                e.dma_start(
                    out=t[:, GUARD + q * qc:GUARD + (q + 1) * qc],
                    in_=x_d[b, hh, :, q * qc:(q + 1) * qc])
            xh[(b, hh)] = t

        load_x(*HIMGS[0], nc.sync, nc.scalar)
        load_x(*HIMGS[1], nc.sync, nc.scalar)

        # ---- compute: pipeline unit = chunk pair (2 chunks, 912 px) ----
        # PE stream software-pipelined: PW of pair k is emitted after DW
        # of pair k+1, so the PE never waits on the y psum->sbuf copies.
        def dw_pair(b, hh, pr):
            xt = xh[(b, hh)]
            pdw = [ps_dw.tile([128, CHUNK], F32, tag="psdw",
                              name=f"dw_{b}_{hh}_{pr}_{c}")
                   for c in range(2)]
            ce = 2 * pr
            for t in range(9):
                ty, tx = t // 3 - 1, t % 3 - 1
                dt = ty * RS + tx
                for ch2 in range(2):             # channel 64-half
                    wsl = wdw_sb[64 * ch2:64 * ch2 + 64,
                                 (hh * 9 + t) * 64:
                                 (hh * 9 + t) * 64 + 64]
                    for par in range(2):         # chunk parity
                        c = ce + par
                        w0 = GUARD + (4 * c + 1) * RS + dt
                        jj = ch2 if par == 0 else 1 - ch2
                        nc.tensor.matmul(
                            pdw[par][64 * jj:64 * jj + 64, :],
                            wsl,
                            xt[64 * ch2:64 * ch2 + 64, w0:w0 + CHUNK],
                            start=(t == 0), stop=(t == 8),
                            skip_group_check=True,
                            tile_position=(64 * ch2, 64 * jj),
                        )
            # y copies (DVE/ACT queues; PE does not wait here)
            ysb = y_pool.tile([128, 2 * CHUNK], BF16, tag="ysb",
                              name=f"ysb_{b}_{hh}_{pr}")
            nc.vector.tensor_copy(ysb[:, 0:CHUNK], pdw[0][:])
            nc.scalar.copy(ysb[:, CHUNK:2 * CHUNK], pdw[1][:])
            return ysb

        # staging covers 2 pairs (one super-chunk) for 3584B DMA runs
        stg_cur = {}

        def pw_pair(b, hh, pr, ysb):
            if pr % 2 == 0:
                stg_cur[0] = [stg_pool.tile([128, 4 * 448], BF16,
                                            tag="stg",
                                            name=f"stg_{b}_{hh}_{pr}_{c}")
                              for c in range(2)]
            stg = stg_cur[0]
            po = (pr % 2) * 2          # pair offset in staging (chunks)
            for par in range(2):
                for co2 in range(2):
                    ppw = ps_pw.tile([128, CHUNK], F32, tag="pspw",
                                     name=f"pw_{b}_{hh}_{pr}_{par}_{co2}")
                    nc.tensor.matmul(
                        ppw[:],
                        wpw_sb[:, ((par * 2 + hh) * 2 + co2) * 128:
                               ((par * 2 + hh) * 2 + co2) * 128 + 128],
                        ysb[:, par * CHUNK:(par + 1) * CHUNK],
                        start=True, stop=True,
                        skip_group_check=True,
                        tile_position=(0, 0),
                    )
                    psrc = ppw[:].rearrange("p (r w) -> p r w", w=RS)
                    pdst = stg[co2][:, (po + par) * 448:
                                    (po + par + 1) * 448
                                    ].rearrange("p (r w) -> p r w", w=112)
                    if (par + co2) % 2 == 0:
                        nc.vector.tensor_copy(pdst, psrc[:, :, 1:113])
                    else:
                        nc.scalar.copy(pdst, psrc[:, :, 1:113])
            last_sc = (b, hh) == HIMGS[-1] and pr >= NPAIR - 2
            if last_sc:
                # tail trim: flush per pair so the final DMAs start early
                sc = pr // 2
                for co2 in range(2):
                    nc.sync.dma_start(
                        out=out_d[b, hh * 256 + co2 * 128:
                                  hh * 256 + co2 * 128 + 128,
                                  sc * 1792 + po * 448:
                                  sc * 1792 + (po + 2) * 448],
                        in_=stg[co2][:, po * 448:(po + 2) * 448])
            elif pr % 2 == 1:
                sc = pr // 2
                for co2 in range(2):
                    nc.sync.dma_start(
                        out=out_d[b, hh * 256 + co2 * 128:
                                  hh * 256 + co2 * 128 + 128,
                                  sc * 1792:(sc + 1) * 1792],
                        in_=stg[co2][:])

        work = [(b, hh, pr) for (b, hh) in HIMGS for pr in range(NPAIR)]
        prev = None
        for wi, (b, hh, pr) in enumerate(work):
            if pr == 5 and (b, hh) != HIMGS[-1]:
                nxt = HIMGS[HIMGS.index((b, hh)) + 1]
                if nxt not in xh:
                    load_x(*nxt, nc.scalar)
            ysb = dw_pair(b, hh, pr)
            if prev is not None:
                pw_pair(*prev)
            prev = (b, hh, pr, ysb)
        pw_pair(*prev)

    nc.compile()
    return nc


def _prep_x(x):
    """(B,H,W,256) fp32 -> (B, 2, 128, 114*114) bf16 padded ch-major."""
    xb = np.asarray(x, dtype=np.float32).astype(ml_dtypes.bfloat16)
    xp = np.zeros((B, 2, 128, NPR, RS), dtype=ml_dtypes.bfloat16)
    xp[:, :, :, 1:113, 1:113] = xb.transpose(0, 3, 1, 2).reshape(
        B, 2, 128, H, W)
    return np.ascontiguousarray(xp.reshape(B, 2, 128, XLEN))


def _prep_weights(splitw, pw):
    sw = np.asarray(splitw, dtype=np.float64)
    pwf = np.asarray(pw, dtype=np.float64).reshape(CPG, COUT)
    diag = sw[:, :, :, np.arange(CPG), np.arange(CPG)]   # (G,3,3,ci)
    # DW: [128 rows, (hh, tap) x 64] diag within each 64-block
    wdw = np.zeros((128, 2 * 9 * 64), dtype=np.float64)
    for p in range(128):
        for hh in range(2):
            ch = hh * 128 + p
            g, ci = ch // 32, ch % 32
            for t in range(9):
                wdw[p, (hh * 9 + t) * 64 + (p % 64)] = \
                    diag[g, t // 3, t % 3, ci]
    # PW: [128 rows, (parity, hh, co2) x 128]
    wpw = np.zeros((128, 2 * 2 * 2 * 128), dtype=np.float64)
    for par in range(2):
        for hh in range(2):
            for co2 in range(2):
                blk = ((par * 2 + hh) * 2 + co2) * 128
                for p in range(128):
                    prow = p if par == 0 else (p + 64) % 128
                    ci_g = hh * 128 + prow
                    for m in range(128):
                        co_g = hh * 256 + co2 * 128 + m
                        if ci_g // 32 == co_g // 64:
                            wpw[p, blk + m] = pwf[ci_g % 32, co_g]
    return (wdw.astype(ml_dtypes.bfloat16), wpw.astype(ml_dtypes.bfloat16))


def _post_out(res_list):
    """8 x (BPC, 512, 12544) bf16 -> (B,H,W,512) fp32."""
    o = np.concatenate(res_list, axis=0)           # (B, 512, 12544)
    return np.ascontiguousarray(
        o.astype(np.float32).transpose(0, 2, 1)).reshape(B, H, W, COUT)


def _in_maps(inputs):
    xt = _prep_x(inputs["x"])
    wdw_arr, wpw_arr = _prep_weights(inputs["splitw"], inputs["pw"])
    return [{"x": xt[i * BPC:(i + 1) * BPC], "w": wdw_arr,
             "wp": wpw_arr}
            for i in range(N_CORES)]


def kernel(x, splitw, pw):
    in_maps = _in_maps({"x": x, "splitw": splitw, "pw": pw})
    if "nc" not in _CACHE:
        _CACHE["nc"] = _build()
    nc = _CACHE["nc"]
    res = run_bass_kernel_spmd(nc, in_maps, list(range(N_CORES)))
    return _post_out([res.results[i]["out"] for i in range(N_CORES)])
